# revision 51
# baseline (speedup 1.0000x reference)
"""Multi-head attention block (B=4, N=1024, C=1024, H=16, d=64) on 8 TRN2 cores.

Sharding: core = 2*b + hh  (batch b in 0..3, head-half hh in 0..1 -> 8 heads/core).
Each core computes the qkv projection for its 8 heads, attention, and a partial
output projection (its 512 rows of w_proj). Host sums the two partials per
batch, rescales, and adds b_proj.

Numerics: the dense projections (QKV, output proj) run as error-compensated
fp8e4 DoubleRow matmuls -- a @ b ~= ah@bh + ah@bl + al@bh with hi = fp8(a),
lo = fp8(a - hi). DoubleRow packs two 128-deep K-tiles per instruction at
0.5 PE cycles per output column, so three compensated passes cost 3/4 of one
fp32r pass. Weights are pre-scaled by 32 on the host so both hi and lo land
in fp8e4's normal range (w ~ N(0, 1/1024) would otherwise put lo terms in
subnormals); the attention-score scale absorbs 1/1024 and the host divides
the output partials by 1024. S^T stays fp32r and the softmax/AV stays bf16:
fp8 probabilities/V measurably exceed the 2e-2 error budget.

Schedule: the ACT exp stream (64 x [128,1024] exps, ~70us) and the PE matmul
stream (~96us) are co-scheduled so neither idles: QKV work is emitted in
per-(strip, seq-half) units and interleaved into the exp windows of earlier
head pairs; AV groups of pair p-1 fill pair p's windows; the cp=0 half of the
output projection runs inside pair 3's windows into a bf16 SBUF accumulator
(atth/attl for pairs 0-1 are ready by then), and the endgame is only AV(3),
the cp=1 projection passes (eviction fused with the accumulator add), and
fp16 output DMA.
"""

import numpy as np

B = 4
N = 1024
C = 1024
H = 16
D = 64
NCORES = 8
SCALE = D ** -0.5
WS = 32.0  # host-side weight prescale so fp8 hi/lo stay in normal range
FP8_PAIR3 = True  # pair-3 attention (heads 6,7 per core) in fp8 DoubleRow AV
ESHIFT = 2.6  # exp shift for fp8 probs: keeps e^(S*scale-ESHIFT) under e4m3 max


_NC_CACHE = {}


def _build_bass():
    import concourse.mybir as mybir
    from concourse import bacc
    from concourse.tile import TileContext

    dt = mybir.dt
    f32 = dt.float32
    f32r = dt.float32r
    f16 = dt.float16
    bf16 = dt.bfloat16
    f8 = dt.float8e4
    Act = mybir.ActivationFunctionType
    DR = mybir.MatmulPerfMode.DoubleRow
    Alu = mybir.AluOpType

    nc = bacc.Bacc(
        "TRN2",
        target_bir_lowering=False,
        debug=False,
        num_devices=NCORES,
        num_swdge_queues=4,
    )

    # ---- DRAM I/O (per-core shards; host prepares fp8 hi/lo layouts) ----
    # Few, large DMAs: HWDGE charges ~625ns + 900ns sem per transfer.
    xh_d = nc.dram_tensor("xh", [128, 4, 2, N], f8, kind="ExternalInput").ap()
    xl_d = nc.dram_tensor("xl", [128, 4, 2, N], f8, kind="ExternalInput").ap()
    wqkh_d = nc.dram_tensor("wqkh", [128, 8, 4, 2, 128], f8, kind="ExternalInput").ap()
    wqkl_d = nc.dram_tensor("wqkl", [128, 8, 4, 2, 128], f8, kind="ExternalInput").ap()
    wvh_d = nc.dram_tensor("wvh", [128, 4, 2, 512], f8, kind="ExternalInput").ap()
    wvl_d = nc.dram_tensor("wvl", [128, 4, 2, 512], f8, kind="ExternalInput").ap()
    wph_d = nc.dram_tensor("wph", [128, 2, 2, C], f8, kind="ExternalInput").ap()
    wpl_d = nc.dram_tensor("wpl", [128, 2, 2, C], f8, kind="ExternalInput").ap()
    bqk_d = nc.dram_tensor("bqk", [128, 8], f32, kind="ExternalInput").ap()
    bv_d = nc.dram_tensor("bv", [128, 512], f32, kind="ExternalInput").ap()
    id_d = nc.dram_tensor("ident", [128, 128], bf16, kind="ExternalInput").ap()
    y_d = nc.dram_tensor("y", [N, C], f16, kind="ExternalOutput").ap()
    # qk column strips stored in first-use order
    ORDER = (0, 4, 1, 5, 2, 6, 3, 7)
    POS = {cc: i for i, cc in enumerate(ORDER)}

    EXP_SCALE = SCALE / (WS * WS)  # q,k carry the 32x weight prescale

    with TileContext(nc) as tc:
        with (
            tc.tile_pool(name="persist", bufs=1) as persist,
            tc.tile_pool(name="yqk_pool", bufs=3) as yqk_pool,
            tc.tile_pool(name="es_pool", bufs=24) as es_pool,
            tc.tile_pool(name="es8_pool", bufs=8) as es8_pool,
            tc.tile_pool(name="norm", bufs=3) as norm,
            tc.tile_pool(name="yo_pool", bufs=3) as yo_pool,
            tc.tile_pool(name="psum", bufs=2, space="PSUM") as ps,
            tc.tile_pool(name="psum_sm", bufs=2, space="PSUM") as ps_sm,
            tc.tile_pool(name="psav", bufs=2, space="PSUM") as psav,
        ):
            # persistent SBUF tensors
            vst = persist.tile([128, 8, 8, 65], bf16, tag="vst")  # [keys128, s, h, d+1]
            # pair-3 V hi/lo in fp8 kc-pair layout [keys128, kp, kt, h2, d+1]
            vh8 = persist.tile([128, 2, 4, 2, 66], f8, tag="vh8")
            vl8 = persist.tile([128, 2, 4, 2, 66], f8, tag="vl8")
            # att'^T hi/lo in proj K-tile-pair layout per (c-chunk pair, q half)
            atth = [
                [
                    persist.tile([128, 2, 512], f8, tag=f"ah{cp}_{qc}", name=f"ah{cp}_{qc}")
                    for qc in range(2)
                ]
                for cp in range(2)
            ]
            attl = [
                [
                    persist.tile([128, 2, 512], f8, tag=f"al{cp}_{qc}", name=f"al{cp}_{qc}")
                    for qc in range(2)
                ]
                for cp in range(2)
            ]
            bqk_t = persist.tile([128, 8], f32, tag="bqk")
            bv_t = persist.tile([128, 512], f32, tag="bv")
            # cp=0 projection partials, accumulated in bf16
            yacc = [
                persist.tile([128, N], bf16, tag=f"yacc{st}", name=f"yacc{st}")
                for st in range(8)
            ]
            # x^T hi/lo [128, kp, ktile, seq]; wqk strips [128, strip, kp, kt, col];
            # wv [128, kp, kt, vcol]; wp [128, cp, kt, outcol] — single tiles,
            # DMA'd in a few large transfers
            xth = persist.tile([128, 4, 2, N], f8, tag="xth")
            xtl = persist.tile([128, 4, 2, N], f8, tag="xtl")
            wqh = persist.tile([128, 8, 4, 2, 128], f8, tag="wqh")
            wql = persist.tile([128, 8, 4, 2, 128], f8, tag="wql")
            wvh = persist.tile([128, 4, 2, 512], f8, tag="wvh")
            wvl = persist.tile([128, 4, 2, 512], f8, tag="wvl")
            wph = persist.tile([128, 2, 2, C], f8, tag="wph")
            wpl = persist.tile([128, 2, 2, C], f8, tag="wpl")
            ident = persist.tile([128, 128], bf16, tag="ident")

            # ones column of V-hat (lo gets 0 so the denominators stay exact)
            nc.vector.memset(vst[:, :, :, 64], 1.0)
            nc.vector.memset(vh8[:, :, :, :, 64:66], 1.0)
            nc.vector.memset(vh8[:, :, :, :, 65], 0.0)
            nc.vector.memset(vl8[:, :, :, :, 64:66], 0.0)
            shift_t = persist.tile([128, 1], f32, tag="shift")
            nc.vector.memset(shift_t[:], -ESHIFT)
            # PE warmup: throwaway matmuls bridge the DMA-paced startup so the
            # p-state ramp never restarts before real work arrives
            wu = persist.tile([128, 128], bf16, tag="wu")
            nc.vector.memset(wu[:], 0.0)
            pwu = psav.tile([66, 512], f32, tag="av", name="pwu")

            def emit_warmup(n):
                for _ in range(n):
                    nc.tensor.matmul(
                        pwu[0:65, 0:128], wu[:, 0:65], wu[:], start=True, stop=True
                    )

            # DMA priority (transfers are serialized): bytes for the pair-0
            # qkv strips and x first, in matmul-pass order, then the rest
            nc.sync.dma_start(wqh[:, 0:2], wqkh_d[:, 0:2])
            nc.sync.dma_start(bqk_t[:], bqk_d)
            nc.sync.dma_start(xth[:, 0:2, :, 0:512], xh_d[:, 0:2, :, 0:512])
            nc.sync.dma_start(xth[:, 2:4, :, 0:512], xh_d[:, 2:4, :, 0:512])
            nc.sync.dma_start(xtl[:, 0:2, :, 0:512], xl_d[:, 0:2, :, 0:512])
            nc.sync.dma_start(xtl[:, 2:4, :, 0:512], xl_d[:, 2:4, :, 0:512])
            nc.sync.dma_start(wql[:, 0:2], wqkl_d[:, 0:2])
            nc.sync.dma_start(wvh[:], wvh_d)
            nc.sync.dma_start(wvl[:], wvl_d)
            nc.sync.dma_start(bv_t[:], bv_d)
            nc.sync.dma_start(xth[:, :, :, 512:1024], xh_d[:, :, :, 512:1024])
            nc.sync.dma_start(xtl[:, :, :, 512:1024], xl_d[:, :, :, 512:1024])
            nc.sync.dma_start(wqh[:, 2:4], wqkh_d[:, 2:4])
            nc.sync.dma_start(wql[:, 2:4], wqkl_d[:, 2:4])
            nc.sync.dma_start(wqh[:, 4:8], wqkh_d[:, 4:8])
            nc.sync.dma_start(wql[:, 4:8], wqkl_d[:, 4:8])
            nc.sync.dma_start(wph[:], wph_d)
            nc.sync.dma_start(wpl[:], wpl_d)
            nc.sync.dma_start(ident[:], id_d)

            # ---- emission units ----
            yqs, yks = {}, {}

            def emit_yqk_unit(p, which, s):
                """One (strip, seq-half) QKV unit: 6 DoubleRow matmuls + evict."""
                cc = p if which == "yq" else 4 + p
                d_ = yqs if which == "yq" else yks
                if p not in d_:
                    d_[p] = yqk_pool.tile([128, N], f32r, tag=which, name=f"{which}{p}")
                yt = d_[p]
                pq = ps_sm.tile([128, 512], f32, tag="sm", name=f"pq{cc}_{s}")
                ci = POS[cc]
                mms = [(wqh, xth), (wqh, xtl), (wql, xth)]
                for mi, (wt, xt) in enumerate(mms):
                    for kp in range(4):
                        nc.tensor.matmul(
                            pq[:],
                            wt[:, ci, kp],
                            xt[:, kp, :, s * 512:(s + 1) * 512],
                            start=(mi == 0 and kp == 0),
                            stop=(mi == 2 and kp == 3),
                            perf_mode=DR,
                        )
                nc.vector.tensor_scalar_add(
                    yt[:, s * 512:(s + 1) * 512], pq[:], bqk_t[:, cc:cc + 1]
                )

            def emit_yv_group(s):
                pv = ps_sm.tile([128, 512], f32, tag="sm", name=f"pv{s}")
                c0 = s * 128
                mms = [(xth, wvh), (xth, wvl), (xtl, wvh)]
                for mi, (xt, wt) in enumerate(mms):
                    for kp in range(4):
                        nc.tensor.matmul(
                            pv[:],
                            xt[:, kp, :, c0:c0 + 128],
                            wt[:, kp],
                            start=(mi == 0 and kp == 0),
                            stop=(mi == 2 and kp == 3),
                            perf_mode=DR,
                        )
                if not FP8_PAIR3:
                    nc.vector.tensor_add(
                        out=vst[:, s, :, 0:64],
                        in0=pv[:].rearrange("p (h d) -> p h d", h=8),
                        in1=bv_t[:].rearrange("p (h d) -> p h d", h=8),
                    )
                    return
                # heads 0-5 stay bf16; heads 6,7 (pair 3) go to fp8 hi/lo
                nc.vector.tensor_add(
                    out=vst[:, s, 0:6, 0:64],
                    in0=pv[:, 0:384].rearrange("p (h d) -> p h d", h=6),
                    in1=bv_t[:, 0:384].rearrange("p (h d) -> p h d", h=6),
                )
                vtmp = norm.tile([128, 128], f32, tag="vtmp", name=f"vtmp{s}")
                nc.vector.tensor_add(
                    out=vtmp[:], in0=pv[:, 384:512], in1=bv_t[:, 384:512]
                )
                kp, kt = s // 2, s % 2
                hi = vh8[:, kt, kp, :, 0:64]
                nc.gpsimd.tensor_copy(hi, vtmp[:].rearrange("p (h d) -> p h d", h=2))
                nc.vector.scalar_tensor_tensor(
                    out=vl8[:, kt, kp, :, 0:64],
                    in0=hi,
                    scalar=-1.0,
                    in1=vtmp[:].rearrange("p (h d) -> p h d", h=2),
                    op0=Alu.mult,
                    op1=Alu.add,
                )

            atf_tiles = {}

            def emit_av_group(p, es, j, qc, alt_psum=False):
                p0 = j * 64
                h = 2 * p + j
                if alt_psum:
                    # endgame: borrow the S psum pool (its buffers free at the
                    # last exp) so the 4 AV(3) groups don't stall on psav's two
                    # buffers behind the normalize chains
                    pav = ps.tile([128, N], f32, tag="s", name=f"pav{h}_{qc}")[0:65, 0:512]
                else:
                    pav = psav.tile([66, 512], f32, tag="av", name=f"pav{h}_{qc}")[0:65, :]
                for kc in range(8):
                    nc.tensor.matmul(
                        pav[:],
                        vst[:, kc, h, :],
                        es[(j, kc)][:, qc * 512:(qc + 1) * 512],
                        start=(kc == 0),
                        stop=(kc == 7),
                    )
                finish_av(p, j, qc, pav)

            def finish_av(p, j, qc, pav):
                p0 = j * 64
                h = 2 * p + j
                rc = norm.tile([1, 512], f32, tag="rc", name=f"rc{h}{qc}")
                nc.vector.reciprocal(rc[:], pav[64:65, :])
                bc = norm.tile([64, 512], f32, tag="bc", name=f"bc{h}{qc}")
                nc.gpsimd.partition_broadcast(bc[:], rc[0:1, :])
                # att' f32 staged full-height so the hi/lo split sees matching
                # SBUF start partitions per head
                if (p, qc) not in atf_tiles:
                    atf_tiles[(p, qc)] = norm.tile(
                        [128, 512], f32, tag="atf", name=f"atf{p}_{qc}"
                    )
                atf = atf_tiles[(p, qc)][p0:p0 + 64, :]
                nc.vector.tensor_mul(out=atf, in0=pav[0:64, :], in1=bc[:])
                cp, i = p // 2, p % 2
                hi = atth[cp][qc][p0:p0 + 64, i, :]
                nc.gpsimd.tensor_copy(hi, atf)
                nc.vector.scalar_tensor_tensor(
                    out=attl[cp][qc][p0:p0 + 64, i, :],
                    in0=hi,
                    scalar=-1.0,
                    in1=atf,
                    op0=Alu.mult,
                    op1=Alu.add,
                )

            def emit_av_group8(es8, j, qc, alt_psum=False):
                """Pair-3 AV in fp8 DoubleRow: es fp8, V hi+lo compensated."""
                h = 6 + j
                if alt_psum:
                    pav = ps.tile([128, N], f32, tag="s", name=f"pav{h}_{qc}")[0:66, 0:512]
                else:
                    pav = psav.tile([66, 512], f32, tag="av", name=f"pav{h}_{qc}")
                for pi, vt in enumerate((vh8, vl8)):
                    for kp in range(4):
                        nc.tensor.matmul(
                            pav[:],
                            vt[:, :, kp, j, :],
                            es8[(j, kp)][:, :, qc * 512:(qc + 1) * 512],
                            start=(pi == 0 and kp == 0),
                            stop=(pi == 1 and kp == 3),
                            perf_mode=DR,
                        )
                finish_av(3, j, qc, pav)

            def emit_proj_a(st, oc, pool_evict):
                """cp=0 partial of the projection -> bf16 accumulator."""
                qc = st // 4
                t0 = (st % 4) * 128
                po = ps_sm.tile([128, 512], f32, tag="sm", name=f"pa{st}_{oc}")
                mms = [(atth, wph), (atth, wpl), (attl, wph)]
                for mi, (at, wt) in enumerate(mms):
                    nc.tensor.matmul(
                        po[:],
                        at[0][qc][:, :, t0:t0 + 128],
                        wt[:, 0, :, oc * 512:(oc + 1) * 512],
                        start=(mi == 0),
                        stop=(mi == 2),
                        perf_mode=DR,
                    )
                nc.vector.tensor_copy(yacc[st][:, oc * 512:(oc + 1) * 512], po[:])

            def emit_proj_b(st):
                """cp=1 passes; the cp=0 partial re-enters PSUM via an
                identity matmul (PE is idle here), eviction on ACT."""
                yo = yo_pool.tile([128, N], f16, tag="yo", name=f"yo{st}")
                qc = st // 4
                t0 = (st % 4) * 128
                for oc in range(2):
                    pool_sel = ps_sm if (st + oc) % 2 == 0 else ps
                    po = pool_sel.tile(
                        [128, 512], f32,
                        tag="sm" if (st + oc) % 2 == 0 else "s",
                        name=f"pb{st}_{oc}",
                    )
                    # identity fold first (only needs yacc), hi terms next,
                    # lo term last: matches hi/lo chain completion order
                    nc.tensor.matmul(
                        po[:],
                        ident[:],
                        yacc[st][:, oc * 512:(oc + 1) * 512],
                        start=True,
                        stop=False,
                    )
                    mms = [(atth, wph), (atth, wpl), (attl, wph)]
                    for mi, (at, wt) in enumerate(mms):
                        nc.tensor.matmul(
                            po[:],
                            at[1][qc][:, :, t0:t0 + 128],
                            wt[:, 1, :, oc * 512:(oc + 1) * 512],
                            start=False,
                            stop=(mi == 2),
                            perf_mode=DR,
                        )
                    if st >= 4 and oc == 1:
                        # late endgame: DVE's normalize chains are drained,
                        # split the eviction load with ACT
                        nc.vector.tensor_copy(yo[:, 512:1024], po[:])
                    else:
                        nc.scalar.copy(yo[:, oc * 512:(oc + 1) * 512], po[:])
                    nc.sync.dma_start(
                        y_d[st * 128:(st + 1) * 128, oc * 512:(oc + 1) * 512],
                        yo[:, oc * 512:(oc + 1) * 512],
                    )

            def emit_st_exp(p, filler, fp8=False):
                """S^T+exp for pair p; `filler(kc)` emits PE work to overlap
                the ACT-paced exp stream. With fp8, probs go to shifted-fp8
                kc-pair tiles for the DoubleRow AV."""
                yq, yk = yqs[p], yks[p]
                es = {}
                for kc in range(8):
                    psj = [
                        ps.tile([128, N], f32, tag="s", name=f"ps{p}_{j}_{kc}")
                        for j in range(2)
                    ]
                    for qc in range(2):
                        for j, p0 in ((0, 0), (1, 64)):
                            nc.tensor.matmul(
                                psj[j][:, qc * 512:(qc + 1) * 512],
                                yk[p0:p0 + 64, kc * 128:(kc + 1) * 128],
                                yq[p0:p0 + 64, qc * 512:(qc + 1) * 512],
                                start=True,
                                stop=True,
                            )
                    for j in range(2):
                        if fp8:
                            kp, kt = kc // 2, kc % 2
                            if (j, kp) not in es:
                                es[(j, kp)] = es8_pool.tile(
                                    [128, 2, N], f8, tag="es8", name=f"e8_{j}_{kp}"
                                )
                            nc.scalar.activation(
                                es[(j, kp)][:, kt, :], psj[j][:], Act.Exp,
                                scale=EXP_SCALE, bias=shift_t[:],
                            )
                        else:
                            e = es_pool.tile(
                                [128, N], bf16, tag="es", name=f"es{p}_{j}_{kc}"
                            )
                            nc.scalar.activation(
                                e[:], psj[j][:], Act.Exp, scale=EXP_SCALE
                            )
                            es[(j, kc)] = e
                    filler(kc)
                return es

            # ---- schedule ----
            def emit_yqk_pair0(s):
                """Pair-0 (strips 0 and 4) for one seq half, passes of the two
                units interleaved to match DMA arrival order."""
                for which in ("yq", "yk"):
                    d_ = yqs if which == "yq" else yks
                    if 0 not in d_:
                        d_[0] = yqk_pool.tile([128, N], f32r, tag=which, name=f"{which}0")
                pqs = {}
                for which in ("yq", "yk"):
                    cc = 0 if which == "yq" else 4
                    pqs[which] = ps_sm.tile([128, 512], f32, tag="sm", name=f"pq{cc}_{s}")
                for mi, (wt, xt) in enumerate([(wqh, xth), (wqh, xtl), (wql, xth)]):
                    for which in ("yq", "yk"):
                        ci = POS[0 if which == "yq" else 4]
                        for kp in range(4):
                            nc.tensor.matmul(
                                pqs[which][:],
                                wt[:, ci, kp],
                                xt[:, kp, :, s * 512:(s + 1) * 512],
                                start=(mi == 0 and kp == 0),
                                stop=(mi == 2 and kp == 3),
                                perf_mode=DR,
                            )
                    if s == 0:
                        # bridge the serialized x-chunk DMA arrivals
                        emit_warmup(3)
                for which in ("yq", "yk"):
                    cc = 0 if which == "yq" else 4
                    yt = (yqs if which == "yq" else yks)[0]
                    nc.vector.tensor_scalar_add(
                        yt[:, s * 512:(s + 1) * 512], pqs[which][:], bqk_t[:, cc:cc + 1]
                    )

            emit_warmup(32)
            emit_yqk_pair0(0)
            emit_yv_group(0)
            emit_yv_group(1)
            emit_yqk_pair0(1)

            def filler0(kc):
                if kc == 0:
                    emit_yv_group(2)
                elif kc == 1:
                    emit_yqk_unit(1, "yq", 0)
                elif kc == 2:
                    emit_yqk_unit(1, "yq", 1)
                    emit_yv_group(3)
                elif kc == 3:
                    emit_yqk_unit(1, "yk", 0)
                    emit_yv_group(4)
                elif kc == 4:
                    emit_yqk_unit(1, "yk", 1)
                    emit_yv_group(5)
                elif kc == 5:
                    emit_yv_group(6)
                elif kc == 6:
                    emit_yv_group(7)

            es_prev = emit_st_exp(0, filler0)
            prev_p = 0
            for p in range(1, 3):
                avq = [(0, 0), (0, 1), (1, 0), (1, 1)]

                def filler(kc, _avq=avq, _pp=prev_p, _es=es_prev, _p=p):
                    if kc % 2 == 1 and _avq:
                        j, qc = _avq.pop(0)
                        emit_av_group(_pp, _es, j, qc)
                    elif kc == 0:
                        emit_yqk_unit(_p + 1, "yq", 0)
                    elif kc == 2:
                        emit_yqk_unit(_p + 1, "yq", 1)
                    elif kc == 4:
                        emit_yqk_unit(_p + 1, "yk", 0)
                    elif kc == 6:
                        emit_yqk_unit(_p + 1, "yk", 1)

                es_now = emit_st_exp(p, filler)
                es_prev, prev_p = es_now, p

            # pair 3: AV(2) on odd windows, cp=0 projection partials on even
            # windows (atth/attl for pairs 0-1 are complete)
            avq = [(0, 0), (0, 1), (1, 0), (1, 1)]
            proj_a_q = [(st, oc) for st in (0, 1, 2, 3, 4, 5, 6, 7) for oc in range(2)]

            def filler3(kc, _avq=avq):
                if kc % 2 == 1 and _avq:
                    j, qc = _avq.pop(0)
                    emit_av_group(2, es_prev, j, qc)
                else:
                    for u in range(4):
                        if proj_a_q:
                            st, oc = proj_a_q.pop(0)
                            emit_proj_a(st, oc, pool_evict=(u % 2 == 0))

            es3 = emit_st_exp(3, filler3, fp8=FP8_PAIR3)
            # ---- endgame: AV(3), then cp=1 projection with fused add ----
            for gi, (j, qc) in enumerate(((0, 0), (1, 0), (0, 1), (1, 1))):
                if FP8_PAIR3:
                    emit_av_group8(es3, j, qc, alt_psum=(gi < 2))
                else:
                    emit_av_group(3, es3, j, qc, alt_psum=(gi < 2))
            for st in range(8):
                emit_proj_b(st)

    nc.compile()
    return nc


def _get_nc():
    if "nc" not in _NC_CACHE:
        _NC_CACHE["nc"] = _build_bass()
    return _NC_CACHE["nc"]


def _hilo(a, e4):
    hi = a.astype(e4)
    lo = (a - hi.astype(np.float32)).astype(e4)
    return hi, lo


def _shard_inputs(x, w_qkv, b_qkv, w_proj):
    """Build per-core input maps. core = 2*b + hh."""
    import ml_dtypes

    e4 = ml_dtypes.float8_e4m3
    bf = ml_dtypes.bfloat16
    ORDER = (0, 4, 1, 5, 2, 6, 3, 7)
    ident = np.eye(128, dtype=bf)

    in_maps = []
    xt_cache = {}
    for core in range(NCORES):
        b = core // 2
        hh = core % 2
        q_sl = slice(hh * 512, (hh + 1) * 512)
        k_sl = slice(1024 + hh * 512, 1024 + (hh + 1) * 512)
        v_sl = slice(2048 + hh * 512, 2048 + (hh + 1) * 512)

        if b not in xt_cache:
            xT = np.ascontiguousarray(x[b].T)
            xp = xT.reshape(4, 2, 128, N).transpose(2, 0, 1, 3)  # [part, kp, kt, N]
            xt_cache[b] = _hilo(np.ascontiguousarray(xp), e4)
        xh, xl = xt_cache[b]

        wqk = np.concatenate([w_qkv[:, q_sl], w_qkv[:, k_sl]], axis=1) * WS
        wqk = wqk.reshape(4, 2, 128, 8, 128).transpose(3, 2, 0, 1, 4)  # [cc, part, kp, kt, col]
        wqk = wqk[list(ORDER)].transpose(1, 0, 2, 3, 4)  # [part, strip, kp, kt, col]
        wqkh, wqkl = _hilo(np.ascontiguousarray(wqk), e4)

        wv = (w_qkv[:, v_sl] * WS).reshape(4, 2, 128, 512).transpose(2, 0, 1, 3)
        wvh, wvl = _hilo(np.ascontiguousarray(wv), e4)

        wp = (w_proj[hh * 512:(hh + 1) * 512, :] * WS).reshape(2, 2, 128, C)
        wp = wp.transpose(2, 0, 1, 3)
        wph, wpl = _hilo(np.ascontiguousarray(wp), e4)

        bqk = np.ascontiguousarray(
            (np.concatenate([b_qkv[q_sl], b_qkv[k_sl]]) * WS).reshape(8, 128).T
        )
        bv = np.ascontiguousarray(np.broadcast_to(b_qkv[v_sl] * WS, (128, 512)))
        in_maps.append(
            {
                "xh": xh,
                "xl": xl,
                "wqkh": wqkh,
                "wqkl": wqkl,
                "wvh": wvh,
                "wvl": wvl,
                "wph": wph,
                "wpl": wpl,
                "bqk": bqk,
                "bv": bv,
                "ident": ident,
            }
        )
    return in_maps


def kernel(x, w_qkv, b_qkv, w_proj, b_proj):
    from concourse.bass_utils import run_bass_kernel_spmd

    x = np.asarray(x, dtype=np.float32)
    w_qkv = np.asarray(w_qkv, dtype=np.float32)
    b_qkv = np.asarray(b_qkv, dtype=np.float32)
    w_proj = np.asarray(w_proj, dtype=np.float32)
    b_proj = np.asarray(b_proj, dtype=np.float32)

    nc = _get_nc()
    in_maps = _shard_inputs(x, w_qkv, b_qkv, w_proj)
    res = run_bass_kernel_spmd(nc, in_maps, core_ids=list(range(NCORES)))

    out = np.empty((B, N, C), dtype=np.float32)
    inv = 1.0 / (WS * WS)  # att' carries 32x from w_qkv', wp' carries 32x
    for b in range(B):
        out[b] = (
            res.results[2 * b]["y"].astype(np.float32)
            + res.results[2 * b + 1]["y"].astype(np.float32)
        ) * inv
    out += b_proj
    return out


# revision 52
# speedup vs baseline: 1.0005x; 1.0005x over previous
"""Multi-head attention block (B=4, N=1024, C=1024, H=16, d=64) on 8 TRN2 cores.

Sharding: core = 2*b + hh  (batch b in 0..3, head-half hh in 0..1 -> 8 heads/core).
Each core computes the qkv projection for its 8 heads, attention, and a partial
output projection (its 512 rows of w_proj). Host sums the two partials per
batch, rescales, and adds b_proj.

Numerics: the dense projections (QKV, output proj) run as error-compensated
fp8e4 DoubleRow matmuls -- a @ b ~= ah@bh + ah@bl + al@bh with hi = fp8(a),
lo = fp8(a - hi). DoubleRow packs two 128-deep K-tiles per instruction at
0.5 PE cycles per output column, so three compensated passes cost 3/4 of one
fp32r pass. Weights are pre-scaled by 32 on the host so both hi and lo land
in fp8e4's normal range (w ~ N(0, 1/1024) would otherwise put lo terms in
subnormals); the attention-score scale absorbs 1/1024 and the host divides
the output partials by 1024. S^T stays fp32r and the softmax/AV stays bf16:
fp8 probabilities/V measurably exceed the 2e-2 error budget.

Schedule: the ACT exp stream (64 x [128,1024] exps, ~70us) and the PE matmul
stream (~96us) are co-scheduled so neither idles: QKV work is emitted in
per-(strip, seq-half) units and interleaved into the exp windows of earlier
head pairs; AV groups of pair p-1 fill pair p's windows; the cp=0 half of the
output projection runs inside pair 3's windows into a bf16 SBUF accumulator
(atth/attl for pairs 0-1 are ready by then), and the endgame is only AV(3),
the cp=1 projection passes (eviction fused with the accumulator add), and
fp16 output DMA.
"""

import numpy as np

B = 4
N = 1024
C = 1024
H = 16
D = 64
NCORES = 8
SCALE = D ** -0.5
WS = 32.0  # host-side weight prescale so fp8 hi/lo stay in normal range
FP8_PAIR3 = False  # pair-3 attention (heads 6,7 per core) in fp8 DoubleRow AV
ESHIFT = 2.6  # exp shift for fp8 probs: keeps e^(S*scale-ESHIFT) under e4m3 max


_NC_CACHE = {}


def _build_bass():
    import concourse.mybir as mybir
    from concourse import bacc
    from concourse.tile import TileContext

    dt = mybir.dt
    f32 = dt.float32
    f32r = dt.float32r
    f16 = dt.float16
    bf16 = dt.bfloat16
    f8 = dt.float8e4
    Act = mybir.ActivationFunctionType
    DR = mybir.MatmulPerfMode.DoubleRow
    Alu = mybir.AluOpType

    nc = bacc.Bacc(
        "TRN2",
        target_bir_lowering=False,
        debug=False,
        num_devices=NCORES,
        num_swdge_queues=4,
    )

    # ---- DRAM I/O (per-core shards; host prepares fp8 hi/lo layouts) ----
    # Few, large DMAs: HWDGE charges ~625ns + 900ns sem per transfer.
    xh_d = nc.dram_tensor("xh", [128, 4, 2, N], f8, kind="ExternalInput").ap()
    xl_d = nc.dram_tensor("xl", [128, 4, 2, N], f8, kind="ExternalInput").ap()
    wqkh_d = nc.dram_tensor("wqkh", [128, 8, 4, 2, 128], f8, kind="ExternalInput").ap()
    wqkl_d = nc.dram_tensor("wqkl", [128, 8, 4, 2, 128], f8, kind="ExternalInput").ap()
    wvh_d = nc.dram_tensor("wvh", [128, 4, 2, 512], f8, kind="ExternalInput").ap()
    wvl_d = nc.dram_tensor("wvl", [128, 4, 2, 512], f8, kind="ExternalInput").ap()
    wph_d = nc.dram_tensor("wph", [128, 2, 2, C], f8, kind="ExternalInput").ap()
    wpl_d = nc.dram_tensor("wpl", [128, 2, 2, C], f8, kind="ExternalInput").ap()
    bqk_d = nc.dram_tensor("bqk", [128, 8], f32, kind="ExternalInput").ap()
    bv_d = nc.dram_tensor("bv", [128, 512], f32, kind="ExternalInput").ap()
    id_d = nc.dram_tensor("ident", [128, 128], bf16, kind="ExternalInput").ap()
    y_d = nc.dram_tensor("y", [N, C], f16, kind="ExternalOutput").ap()
    # qk column strips stored in first-use order
    ORDER = (0, 4, 1, 5, 2, 6, 3, 7)
    POS = {cc: i for i, cc in enumerate(ORDER)}

    EXP_SCALE = SCALE / (WS * WS)  # q,k carry the 32x weight prescale

    with TileContext(nc) as tc:
        with (
            tc.tile_pool(name="persist", bufs=1) as persist,
            tc.tile_pool(name="yqk_pool", bufs=3) as yqk_pool,
            tc.tile_pool(name="es_pool", bufs=24) as es_pool,
            tc.tile_pool(name="es8_pool", bufs=8) as es8_pool,
            tc.tile_pool(name="norm", bufs=3) as norm,
            tc.tile_pool(name="yo_pool", bufs=3) as yo_pool,
            tc.tile_pool(name="psum", bufs=2, space="PSUM") as ps,
            tc.tile_pool(name="psum_sm", bufs=2, space="PSUM") as ps_sm,
            tc.tile_pool(name="psav", bufs=2, space="PSUM") as psav,
        ):
            # persistent SBUF tensors
            vst = persist.tile([128, 8, 8, 65], bf16, tag="vst")  # [keys128, s, h, d+1]
            # pair-3 V hi/lo in fp8 kc-pair layout [keys128, kp, kt, h2, d+1]
            vh8 = persist.tile([128, 2, 4, 2, 66], f8, tag="vh8")
            vl8 = persist.tile([128, 2, 4, 2, 66], f8, tag="vl8")
            # att'^T hi/lo in proj K-tile-pair layout per (c-chunk pair, q half)
            atth = [
                [
                    persist.tile([128, 2, 512], f8, tag=f"ah{cp}_{qc}", name=f"ah{cp}_{qc}")
                    for qc in range(2)
                ]
                for cp in range(2)
            ]
            attl = [
                [
                    persist.tile([128, 2, 512], f8, tag=f"al{cp}_{qc}", name=f"al{cp}_{qc}")
                    for qc in range(2)
                ]
                for cp in range(2)
            ]
            bqk_t = persist.tile([128, 8], f32, tag="bqk")
            bv_t = persist.tile([128, 512], f32, tag="bv")
            # cp=0 projection partials, accumulated in bf16
            yacc = [
                persist.tile([128, N], bf16, tag=f"yacc{st}", name=f"yacc{st}")
                for st in range(8)
            ]
            # x^T hi/lo [128, kp, ktile, seq]; wqk strips [128, strip, kp, kt, col];
            # wv [128, kp, kt, vcol]; wp [128, cp, kt, outcol] — single tiles,
            # DMA'd in a few large transfers
            xth = persist.tile([128, 4, 2, N], f8, tag="xth")
            xtl = persist.tile([128, 4, 2, N], f8, tag="xtl")
            wqh = persist.tile([128, 8, 4, 2, 128], f8, tag="wqh")
            wql = persist.tile([128, 8, 4, 2, 128], f8, tag="wql")
            wvh = persist.tile([128, 4, 2, 512], f8, tag="wvh")
            wvl = persist.tile([128, 4, 2, 512], f8, tag="wvl")
            wph = persist.tile([128, 2, 2, C], f8, tag="wph")
            wpl = persist.tile([128, 2, 2, C], f8, tag="wpl")
            ident = persist.tile([128, 128], bf16, tag="ident")

            # ones column of V-hat (lo gets 0 so the denominators stay exact)
            nc.vector.memset(vst[:, :, :, 64], 1.0)
            if FP8_PAIR3:
                nc.vector.memset(vh8[:, :, :, :, 64:66], 1.0)
                nc.vector.memset(vh8[:, :, :, :, 65], 0.0)
                nc.vector.memset(vl8[:, :, :, :, 64:66], 0.0)
            shift_t = persist.tile([128, 1], f32, tag="shift")
            nc.vector.memset(shift_t[:], -ESHIFT)
            # PE warmup: throwaway matmuls bridge the DMA-paced startup so the
            # p-state ramp never restarts before real work arrives
            wu = persist.tile([128, 128], bf16, tag="wu")
            nc.vector.memset(wu[:], 0.0)
            pwu = psav.tile([66, 512], f32, tag="av", name="pwu")

            def emit_warmup(n):
                for _ in range(n):
                    nc.tensor.matmul(
                        pwu[0:65, 0:128], wu[:, 0:65], wu[:], start=True, stop=True
                    )

            # DMA priority (transfers are serialized): bytes for the pair-0
            # qkv strips and x first, in matmul-pass order, then the rest
            nc.sync.dma_start(wqh[:, 0:2], wqkh_d[:, 0:2])
            nc.sync.dma_start(bqk_t[:], bqk_d)
            nc.sync.dma_start(xth[:, 0:2, :, 0:512], xh_d[:, 0:2, :, 0:512])
            nc.sync.dma_start(xth[:, 2:4, :, 0:512], xh_d[:, 2:4, :, 0:512])
            nc.sync.dma_start(xtl[:, 0:2, :, 0:512], xl_d[:, 0:2, :, 0:512])
            nc.sync.dma_start(xtl[:, 2:4, :, 0:512], xl_d[:, 2:4, :, 0:512])
            nc.sync.dma_start(wql[:, 0:2], wqkl_d[:, 0:2])
            nc.sync.dma_start(wvh[:], wvh_d)
            nc.sync.dma_start(wvl[:], wvl_d)
            nc.sync.dma_start(bv_t[:], bv_d)
            nc.sync.dma_start(xth[:, :, :, 512:1024], xh_d[:, :, :, 512:1024])
            nc.sync.dma_start(xtl[:, :, :, 512:1024], xl_d[:, :, :, 512:1024])
            nc.sync.dma_start(wqh[:, 2:4], wqkh_d[:, 2:4])
            nc.sync.dma_start(wql[:, 2:4], wqkl_d[:, 2:4])
            nc.sync.dma_start(wqh[:, 4:8], wqkh_d[:, 4:8])
            nc.sync.dma_start(wql[:, 4:8], wqkl_d[:, 4:8])
            nc.sync.dma_start(wph[:], wph_d)
            nc.sync.dma_start(wpl[:], wpl_d)
            nc.sync.dma_start(ident[:], id_d)

            # ---- emission units ----
            yqs, yks = {}, {}

            def emit_yqk_unit(p, which, s):
                """One (strip, seq-half) QKV unit: 6 DoubleRow matmuls + evict."""
                cc = p if which == "yq" else 4 + p
                d_ = yqs if which == "yq" else yks
                if p not in d_:
                    d_[p] = yqk_pool.tile([128, N], f32r, tag=which, name=f"{which}{p}")
                yt = d_[p]
                pq = ps_sm.tile([128, 512], f32, tag="sm", name=f"pq{cc}_{s}")
                ci = POS[cc]
                mms = [(wqh, xth), (wqh, xtl), (wql, xth)]
                for mi, (wt, xt) in enumerate(mms):
                    for kp in range(4):
                        nc.tensor.matmul(
                            pq[:],
                            wt[:, ci, kp],
                            xt[:, kp, :, s * 512:(s + 1) * 512],
                            start=(mi == 0 and kp == 0),
                            stop=(mi == 2 and kp == 3),
                            perf_mode=DR,
                        )
                nc.vector.tensor_scalar_add(
                    yt[:, s * 512:(s + 1) * 512], pq[:], bqk_t[:, cc:cc + 1]
                )

            def emit_yv_group(s):
                pv = ps_sm.tile([128, 512], f32, tag="sm", name=f"pv{s}")
                c0 = s * 128
                mms = [(xth, wvh), (xth, wvl), (xtl, wvh)]
                for mi, (xt, wt) in enumerate(mms):
                    for kp in range(4):
                        nc.tensor.matmul(
                            pv[:],
                            xt[:, kp, :, c0:c0 + 128],
                            wt[:, kp],
                            start=(mi == 0 and kp == 0),
                            stop=(mi == 2 and kp == 3),
                            perf_mode=DR,
                        )
                if not FP8_PAIR3:
                    nc.vector.tensor_add(
                        out=vst[:, s, :, 0:64],
                        in0=pv[:].rearrange("p (h d) -> p h d", h=8),
                        in1=bv_t[:].rearrange("p (h d) -> p h d", h=8),
                    )
                    return
                # heads 0-5 stay bf16; heads 6,7 (pair 3) go to fp8 hi/lo
                nc.vector.tensor_add(
                    out=vst[:, s, 0:6, 0:64],
                    in0=pv[:, 0:384].rearrange("p (h d) -> p h d", h=6),
                    in1=bv_t[:, 0:384].rearrange("p (h d) -> p h d", h=6),
                )
                vtmp = norm.tile([128, 128], f32, tag="vtmp", name=f"vtmp{s}")
                nc.vector.tensor_add(
                    out=vtmp[:], in0=pv[:, 384:512], in1=bv_t[:, 384:512]
                )
                kp, kt = s // 2, s % 2
                hi = vh8[:, kt, kp, :, 0:64]
                nc.gpsimd.tensor_copy(hi, vtmp[:].rearrange("p (h d) -> p h d", h=2))
                nc.vector.scalar_tensor_tensor(
                    out=vl8[:, kt, kp, :, 0:64],
                    in0=hi,
                    scalar=-1.0,
                    in1=vtmp[:].rearrange("p (h d) -> p h d", h=2),
                    op0=Alu.mult,
                    op1=Alu.add,
                )

            atf_tiles = {}

            def emit_av_group(p, es, j, qc, alt_psum=False):
                p0 = j * 64
                h = 2 * p + j
                if alt_psum:
                    # endgame: borrow the S psum pool (its buffers free at the
                    # last exp) so the 4 AV(3) groups don't stall on psav's two
                    # buffers behind the normalize chains
                    pav = ps.tile([128, N], f32, tag="s", name=f"pav{h}_{qc}")[0:65, 0:512]
                else:
                    pav = psav.tile([66, 512], f32, tag="av", name=f"pav{h}_{qc}")[0:65, :]
                for kc in range(8):
                    nc.tensor.matmul(
                        pav[:],
                        vst[:, kc, h, :],
                        es[(j, kc)][:, qc * 512:(qc + 1) * 512],
                        start=(kc == 0),
                        stop=(kc == 7),
                    )
                finish_av(p, j, qc, pav)

            def finish_av(p, j, qc, pav):
                p0 = j * 64
                h = 2 * p + j
                rc = norm.tile([1, 512], f32, tag="rc", name=f"rc{h}{qc}")
                nc.vector.reciprocal(rc[:], pav[64:65, :])
                bc = norm.tile([64, 512], f32, tag="bc", name=f"bc{h}{qc}")
                nc.gpsimd.partition_broadcast(bc[:], rc[0:1, :])
                # att' f32 staged full-height so the hi/lo split sees matching
                # SBUF start partitions per head
                if (p, qc) not in atf_tiles:
                    atf_tiles[(p, qc)] = norm.tile(
                        [128, 512], f32, tag="atf", name=f"atf{p}_{qc}"
                    )
                atf = atf_tiles[(p, qc)][p0:p0 + 64, :]
                nc.vector.tensor_mul(out=atf, in0=pav[0:64, :], in1=bc[:])
                cp, i = p // 2, p % 2
                hi = atth[cp][qc][p0:p0 + 64, i, :]
                nc.gpsimd.tensor_copy(hi, atf)
                nc.vector.scalar_tensor_tensor(
                    out=attl[cp][qc][p0:p0 + 64, i, :],
                    in0=hi,
                    scalar=-1.0,
                    in1=atf,
                    op0=Alu.mult,
                    op1=Alu.add,
                )

            def emit_av_group8(es8, j, qc, alt_psum=False):
                """Pair-3 AV in fp8 DoubleRow: es fp8, V hi+lo compensated."""
                h = 6 + j
                if alt_psum:
                    pav = ps.tile([128, N], f32, tag="s", name=f"pav{h}_{qc}")[0:66, 0:512]
                else:
                    pav = psav.tile([66, 512], f32, tag="av", name=f"pav{h}_{qc}")
                for pi, vt in enumerate((vh8, vl8)):
                    for kp in range(4):
                        nc.tensor.matmul(
                            pav[:],
                            vt[:, :, kp, j, :],
                            es8[(j, kp)][:, :, qc * 512:(qc + 1) * 512],
                            start=(pi == 0 and kp == 0),
                            stop=(pi == 1 and kp == 3),
                            perf_mode=DR,
                        )
                finish_av(3, j, qc, pav)

            def emit_proj_a(st, oc, pool_evict):
                """cp=0 partial of the projection -> bf16 accumulator."""
                qc = st // 4
                t0 = (st % 4) * 128
                po = ps_sm.tile([128, 512], f32, tag="sm", name=f"pa{st}_{oc}")
                mms = [(atth, wph), (atth, wpl), (attl, wph)]
                for mi, (at, wt) in enumerate(mms):
                    nc.tensor.matmul(
                        po[:],
                        at[0][qc][:, :, t0:t0 + 128],
                        wt[:, 0, :, oc * 512:(oc + 1) * 512],
                        start=(mi == 0),
                        stop=(mi == 2),
                        perf_mode=DR,
                    )
                nc.vector.tensor_copy(yacc[st][:, oc * 512:(oc + 1) * 512], po[:])

            def emit_proj_b(st):
                """cp=1 passes; the cp=0 partial re-enters PSUM via an
                identity matmul (PE is idle here), eviction on ACT."""
                yo = yo_pool.tile([128, N], f16, tag="yo", name=f"yo{st}")
                qc = st // 4
                t0 = (st % 4) * 128
                for oc in range(2):
                    pool_sel = ps_sm if (st + oc) % 2 == 0 else ps
                    po = pool_sel.tile(
                        [128, 512], f32,
                        tag="sm" if (st + oc) % 2 == 0 else "s",
                        name=f"pb{st}_{oc}",
                    )
                    # identity fold first (only needs yacc), hi terms next,
                    # lo term last: matches hi/lo chain completion order
                    nc.tensor.matmul(
                        po[:],
                        ident[:],
                        yacc[st][:, oc * 512:(oc + 1) * 512],
                        start=True,
                        stop=False,
                    )
                    mms = [(atth, wph), (atth, wpl), (attl, wph)]
                    for mi, (at, wt) in enumerate(mms):
                        nc.tensor.matmul(
                            po[:],
                            at[1][qc][:, :, t0:t0 + 128],
                            wt[:, 1, :, oc * 512:(oc + 1) * 512],
                            start=False,
                            stop=(mi == 2),
                            perf_mode=DR,
                        )
                    if st >= 4 and oc == 1:
                        # late endgame: DVE's normalize chains are drained,
                        # split the eviction load with ACT
                        nc.vector.tensor_copy(yo[:, 512:1024], po[:])
                    else:
                        nc.scalar.copy(yo[:, oc * 512:(oc + 1) * 512], po[:])
                    nc.sync.dma_start(
                        y_d[st * 128:(st + 1) * 128, oc * 512:(oc + 1) * 512],
                        yo[:, oc * 512:(oc + 1) * 512],
                    )

            def emit_st_exp(p, filler, fp8=False):
                """S^T+exp for pair p; `filler(kc)` emits PE work to overlap
                the ACT-paced exp stream. With fp8, probs go to shifted-fp8
                kc-pair tiles for the DoubleRow AV."""
                yq, yk = yqs[p], yks[p]
                es = {}
                for kc in range(8):
                    psj = [
                        ps.tile([128, N], f32, tag="s", name=f"ps{p}_{j}_{kc}")
                        for j in range(2)
                    ]
                    for qc in range(2):
                        for j, p0 in ((0, 0), (1, 64)):
                            nc.tensor.matmul(
                                psj[j][:, qc * 512:(qc + 1) * 512],
                                yk[p0:p0 + 64, kc * 128:(kc + 1) * 128],
                                yq[p0:p0 + 64, qc * 512:(qc + 1) * 512],
                                start=True,
                                stop=True,
                            )
                    for j in range(2):
                        if fp8:
                            kp, kt = kc // 2, kc % 2
                            if (j, kp) not in es:
                                es[(j, kp)] = es8_pool.tile(
                                    [128, 2, N], f8, tag="es8", name=f"e8_{j}_{kp}"
                                )
                            nc.scalar.activation(
                                es[(j, kp)][:, kt, :], psj[j][:], Act.Exp,
                                scale=EXP_SCALE, bias=shift_t[:],
                            )
                        else:
                            e = es_pool.tile(
                                [128, N], bf16, tag="es", name=f"es{p}_{j}_{kc}"
                            )
                            nc.scalar.activation(
                                e[:], psj[j][:], Act.Exp, scale=EXP_SCALE
                            )
                            es[(j, kc)] = e
                    filler(kc)
                return es

            # ---- schedule ----
            def emit_yqk_pair0(s):
                """Pair-0 (strips 0 and 4) for one seq half, passes of the two
                units interleaved to match DMA arrival order."""
                for which in ("yq", "yk"):
                    d_ = yqs if which == "yq" else yks
                    if 0 not in d_:
                        d_[0] = yqk_pool.tile([128, N], f32r, tag=which, name=f"{which}0")
                pqs = {}
                for which in ("yq", "yk"):
                    cc = 0 if which == "yq" else 4
                    pqs[which] = ps_sm.tile([128, 512], f32, tag="sm", name=f"pq{cc}_{s}")
                for mi, (wt, xt) in enumerate([(wqh, xth), (wqh, xtl), (wql, xth)]):
                    for which in ("yq", "yk"):
                        ci = POS[0 if which == "yq" else 4]
                        for kp in range(4):
                            nc.tensor.matmul(
                                pqs[which][:],
                                wt[:, ci, kp],
                                xt[:, kp, :, s * 512:(s + 1) * 512],
                                start=(mi == 0 and kp == 0),
                                stop=(mi == 2 and kp == 3),
                                perf_mode=DR,
                            )
                    if s == 0:
                        # bridge the serialized x-chunk DMA arrivals
                        emit_warmup(3)
                for which in ("yq", "yk"):
                    cc = 0 if which == "yq" else 4
                    yt = (yqs if which == "yq" else yks)[0]
                    nc.vector.tensor_scalar_add(
                        yt[:, s * 512:(s + 1) * 512], pqs[which][:], bqk_t[:, cc:cc + 1]
                    )

            emit_warmup(32)
            emit_yqk_pair0(0)
            emit_yv_group(0)
            emit_yv_group(1)
            emit_yqk_pair0(1)

            def filler0(kc):
                if kc == 0:
                    emit_yv_group(2)
                elif kc == 1:
                    emit_yqk_unit(1, "yq", 0)
                elif kc == 2:
                    emit_yqk_unit(1, "yq", 1)
                    emit_yv_group(3)
                elif kc == 3:
                    emit_yqk_unit(1, "yk", 0)
                    emit_yv_group(4)
                elif kc == 4:
                    emit_yqk_unit(1, "yk", 1)
                    emit_yv_group(5)
                elif kc == 5:
                    emit_yv_group(6)
                elif kc == 6:
                    emit_yv_group(7)

            es_prev = emit_st_exp(0, filler0)
            prev_p = 0
            for p in range(1, 3):
                avq = [(0, 0), (0, 1), (1, 0), (1, 1)]

                def filler(kc, _avq=avq, _pp=prev_p, _es=es_prev, _p=p):
                    if kc % 2 == 1 and _avq:
                        j, qc = _avq.pop(0)
                        emit_av_group(_pp, _es, j, qc)
                    elif kc == 0:
                        emit_yqk_unit(_p + 1, "yq", 0)
                    elif kc == 2:
                        emit_yqk_unit(_p + 1, "yq", 1)
                    elif kc == 4:
                        emit_yqk_unit(_p + 1, "yk", 0)
                    elif kc == 6:
                        emit_yqk_unit(_p + 1, "yk", 1)

                es_now = emit_st_exp(p, filler)
                es_prev, prev_p = es_now, p

            # pair 3: AV(2) on odd windows, cp=0 projection partials on even
            # windows (atth/attl for pairs 0-1 are complete)
            avq = [(0, 0), (0, 1), (1, 0), (1, 1)]
            proj_a_q = [(st, oc) for st in (0, 1, 2, 3, 4, 5, 6, 7) for oc in range(2)]

            def filler3(kc, _avq=avq):
                if kc % 2 == 1 and _avq:
                    j, qc = _avq.pop(0)
                    emit_av_group(2, es_prev, j, qc)
                else:
                    for u in range(4):
                        if proj_a_q:
                            st, oc = proj_a_q.pop(0)
                            emit_proj_a(st, oc, pool_evict=(u % 2 == 0))

            es3 = emit_st_exp(3, filler3, fp8=FP8_PAIR3)
            # ---- endgame: AV(3), then cp=1 projection with fused add ----
            for gi, (j, qc) in enumerate(((0, 0), (1, 0), (0, 1), (1, 1))):
                if FP8_PAIR3:
                    emit_av_group8(es3, j, qc, alt_psum=(gi < 2))
                else:
                    emit_av_group(3, es3, j, qc, alt_psum=(gi < 2))
            for st in range(8):
                emit_proj_b(st)

    nc.compile()
    return nc


def _get_nc():
    if "nc" not in _NC_CACHE:
        _NC_CACHE["nc"] = _build_bass()
    return _NC_CACHE["nc"]


def _hilo(a, e4):
    hi = a.astype(e4)
    lo = (a - hi.astype(np.float32)).astype(e4)
    return hi, lo


def _shard_inputs(x, w_qkv, b_qkv, w_proj):
    """Build per-core input maps. core = 2*b + hh."""
    import ml_dtypes

    e4 = ml_dtypes.float8_e4m3
    bf = ml_dtypes.bfloat16
    ORDER = (0, 4, 1, 5, 2, 6, 3, 7)
    ident = np.eye(128, dtype=bf)

    in_maps = []
    xt_cache = {}
    for core in range(NCORES):
        b = core // 2
        hh = core % 2
        q_sl = slice(hh * 512, (hh + 1) * 512)
        k_sl = slice(1024 + hh * 512, 1024 + (hh + 1) * 512)
        v_sl = slice(2048 + hh * 512, 2048 + (hh + 1) * 512)

        if b not in xt_cache:
            xT = np.ascontiguousarray(x[b].T)
            xp = xT.reshape(4, 2, 128, N).transpose(2, 0, 1, 3)  # [part, kp, kt, N]
            xt_cache[b] = _hilo(np.ascontiguousarray(xp), e4)
        xh, xl = xt_cache[b]

        wqk = np.concatenate([w_qkv[:, q_sl], w_qkv[:, k_sl]], axis=1) * WS
        wqk = wqk.reshape(4, 2, 128, 8, 128).transpose(3, 2, 0, 1, 4)  # [cc, part, kp, kt, col]
        wqk = wqk[list(ORDER)].transpose(1, 0, 2, 3, 4)  # [part, strip, kp, kt, col]
        wqkh, wqkl = _hilo(np.ascontiguousarray(wqk), e4)

        wv = (w_qkv[:, v_sl] * WS).reshape(4, 2, 128, 512).transpose(2, 0, 1, 3)
        wvh, wvl = _hilo(np.ascontiguousarray(wv), e4)

        wp = (w_proj[hh * 512:(hh + 1) * 512, :] * WS).reshape(2, 2, 128, C)
        wp = wp.transpose(2, 0, 1, 3)
        wph, wpl = _hilo(np.ascontiguousarray(wp), e4)

        bqk = np.ascontiguousarray(
            (np.concatenate([b_qkv[q_sl], b_qkv[k_sl]]) * WS).reshape(8, 128).T
        )
        bv = np.ascontiguousarray(np.broadcast_to(b_qkv[v_sl] * WS, (128, 512)))
        in_maps.append(
            {
                "xh": xh,
                "xl": xl,
                "wqkh": wqkh,
                "wqkl": wqkl,
                "wvh": wvh,
                "wvl": wvl,
                "wph": wph,
                "wpl": wpl,
                "bqk": bqk,
                "bv": bv,
                "ident": ident,
            }
        )
    return in_maps


def kernel(x, w_qkv, b_qkv, w_proj, b_proj):
    from concourse.bass_utils import run_bass_kernel_spmd

    x = np.asarray(x, dtype=np.float32)
    w_qkv = np.asarray(w_qkv, dtype=np.float32)
    b_qkv = np.asarray(b_qkv, dtype=np.float32)
    w_proj = np.asarray(w_proj, dtype=np.float32)
    b_proj = np.asarray(b_proj, dtype=np.float32)

    nc = _get_nc()
    in_maps = _shard_inputs(x, w_qkv, b_qkv, w_proj)
    res = run_bass_kernel_spmd(nc, in_maps, core_ids=list(range(NCORES)))

    out = np.empty((B, N, C), dtype=np.float32)
    inv = 1.0 / (WS * WS)  # att' carries 32x from w_qkv', wp' carries 32x
    for b in range(B):
        out[b] = (
            res.results[2 * b]["y"].astype(np.float32)
            + res.results[2 * b + 1]["y"].astype(np.float32)
        ) * inv
    out += b_proj
    return out
